# revision 27
# baseline (speedup 1.0000x reference)
"""Trainium2 Bass kernel for nn_Attention_53257594471037.

Multi-head attention layer (B=8, N=1024, embed 512 + class 512):
  qk = x[:, :, -512:] @ Wqk + bqk ; q, k = split(qk)      (8 heads, dh=64)
  v  = x @ Wv + bv                                        (8 heads, dv=128)
  out = softmax(q k^T / sqrt(64)) v                       per head
  y  = concat(out) @ Wo + bo

Sharding: data-parallel over batch — each of the 8 NeuronCores handles one
batch element end to end.  No collectives.

Per-core plan (all matmuls in bf16, fp32 accumulation in PSUM):
  - x is DMA'd (cast to bf16) and transposed on the PE into xT [feat, tok];
    the class half is loaded and transposed first since only it feeds the
    qk projection.
  - qkT[f, n] = Wqk^T @ x_clsT + bqk computed directly in transposed layout,
    which gives q^T / k^T per head ([64, 1024] slices) for free.
  - S^T[j, i] per head has j on partitions so softmax-exp runs on ACT
    straight out of PSUM; two heads are packed into the PE at once
    (K=64 row tiling).
  - The softmax denominator comes free from the PV matmul: V is augmented
    with a ones column, so out_psum[:, 128] = sum_j exp(S^T[j, i]).
  - bv is NOT added to v: softmax rows sum to one, so the bias passes
    through attention unchanged and is added per-partition after the
    out -> outT transpose instead (where vfeat sits on partitions).
  - y = outT^T @ Wo + bo.

Emission interleaves phases so the PE never waits on ACT exp:
v-projection steps fill QKT gaps of pairs 0-2, PV of pair c-1 fills the
QKT gaps of pair c, out-transposes of heads 0-3 ride with pair 3, and the
remaining out-transposes alternate with y-projection steps.
"""

import os

os.environ.setdefault("MYCRO_LOCAL_CACHE", "1")

import numpy as np

# --- problem constants (hardcoded; kernel.py must be self-contained) ---
B = 8
N = 1024          # tokens
D = 1024          # embed + class feature width
CLS = 512         # class width; qk projection reads x[:, :, -CLS:]
HEADS = 8
DH = 64           # per-head q/k dim
DV = 128          # per-head v dim
SCALE = DH ** -0.5
NT = N // 128     # 8 token tiles
DC = D // 128     # 8 feature chunks
VSTRIDE = 130     # per-head stride in v_aug: 128 data + 1 ones + 1 pad

_COMPILED = None  # cached compiled module so repeated kernel() calls reuse it


def _build():
    import concourse.mybir as mybir
    import concourse.tile as tile
    from concourse import bacc

    f32 = mybir.dt.float32
    bf16 = mybir.dt.bfloat16
    Exp = mybir.ActivationFunctionType.Exp
    Ident = mybir.ActivationFunctionType.Identity
    mult = mybir.AluOpType.mult
    add = mybir.AluOpType.add

    nc = bacc.Bacc(None, target_bir_lowering=False)

    x_d = nc.declare_dram_parameter("x", [N, D], f32, isOutput=False)
    wqk_d = nc.declare_dram_parameter("Wqk", [CLS, 2 * HEADS * DH], f32, isOutput=False)
    bqk_d = nc.declare_dram_parameter("bqk", [2 * HEADS * DH], f32, isOutput=False)
    wv_d = nc.declare_dram_parameter("Wv", [D, D], f32, isOutput=False)
    bv_d = nc.declare_dram_parameter("bv", [D], f32, isOutput=False)
    wo_d = nc.declare_dram_parameter("Wo", [D, D], f32, isOutput=False)
    bo_d = nc.declare_dram_parameter("bo", [D], f32, isOutput=False)
    y_d = nc.declare_dram_parameter("y", [N, D], f32, isOutput=True)

    ident_const = nc.inline_tensor(
        np.eye(128, dtype=np.float32).astype(__import__("ml_dtypes").bfloat16),
        name="identc",
    )

    with tile.TileContext(nc) as tc:
        with (
            tc.tile_pool(name="persist", bufs=1) as pp,
            tc.tile_pool(name="xin", bufs=4) as xp,
            tc.tile_pool(name="expsp", bufs=4) as ep,
            tc.tile_pool(name="small", bufs=2) as sp,
            tc.tile_pool(name="yout", bufs=2) as yp,
            tc.tile_pool(name="ps_mm", bufs=2, space="PSUM") as ps_mm,
            tc.tile_pool(name="ps_s", bufs=2, space="PSUM") as ps_s,
            tc.tile_pool(name="ps_o", bufs=2, space="PSUM") as ps_o,
        ):
            # ---------- loads; gpsimd (SWDGE, casting) queue order matters ----
            # Few LARGE DMAs: each gpsimd dma_start costs ~1us of Q7 descriptor
            # work and the queue is FIFO, so x rides in four 1MB tiles.  The
            # class half goes first (it alone feeds the qk projection); the
            # embed half loads LAST so its wait-for-slot (it reuses the class
            # tiles' slots) never stalls the weight loads behind it.
            x_r = x_d.rearrange("(t p) f -> p t f", p=128)
            xc_tiles, xe_tiles = [], []
            xc = xp.tile([128, 4, CLS], bf16, tag="xbig", name="xc0", bufs=4)
            nc.gpsimd.dma_start(out=xc[:, 0:1, :], in_=x_r[:, 0:1, CLS:D])
            nc.gpsimd.dma_start(out=xc[:, 1:4, :], in_=x_r[:, 1:4, CLS:D])
            xc_tiles.append(xc)
            wqk_sb = pp.tile([128, CLS // 128, 1024], bf16, tag="big2")
            nc.gpsimd.dma_start(
                out=wqk_sb[:, :, :], in_=wqk_d.rearrange("(c p) n -> p c n", p=128)
            )
            xc = xp.tile([128, 4, CLS], bf16, tag="xbig", name="xc1", bufs=4)
            nc.gpsimd.dma_start(out=xc[:, :, :], in_=x_r[:, 4:8, CLS:D])
            xc_tiles.append(xc)
            xe0 = xp.tile([128, 4, CLS], bf16, tag="xbig", name="xe0", bufs=4)
            nc.gpsimd.dma_start(out=xe0[:, :, :], in_=x_r[:, 0:4, 0:CLS])
            xe_tiles.append(xe0)
            wv_sb = pp.tile([128, DC, 1024], bf16)
            nc.gpsimd.dma_start(
                out=wv_sb[:, :, 0:512],
                in_=wv_d.rearrange("(c p) n -> p c n", p=128)[:, :, 0:512],
            )
            xe1 = xp.tile([128, 4, CLS], bf16, tag="xbig", name="xe1", bufs=4)
            nc.gpsimd.dma_start(out=xe1[:, :, :], in_=x_r[:, 4:8, 0:CLS])
            xe_tiles.append(xe1)
            nc.gpsimd.dma_start(
                out=wv_sb[:, :, 512:1024],
                in_=wv_d.rearrange("(c p) n -> p c n", p=128)[:, :, 512:1024],
            )
            wo_sb = pp.tile([128, DC, 1024], bf16)
            nc.gpsimd.dma_start(
                out=wo_sb[:, :, :], in_=wo_d.rearrange("(c p) n -> p c n", p=128)
            )
            # biases + identity ride the HWDGE queue, parallel to the above
            ident = pp.tile([128, 128], bf16)
            nc.sync.dma_start(out=ident[:, :], in_=ident_const[:, :])
            bqk_col = pp.tile([128, 8], f32)
            nc.sync.dma_start(
                out=bqk_col[:, :], in_=bqk_d.rearrange("(m p) -> p m", p=128)
            )
            bv_col = pp.tile([128, 8], f32)
            nc.sync.dma_start(
                out=bv_col[:, :], in_=bv_d.rearrange("(m p) -> p m", p=128)
            )
            bo_bc = pp.tile([128, D], bf16)
            nc.gpsimd.dma_start(
                out=bo_bc[:, :],
                in_=bo_d.rearrange("(o n) -> o n", o=1).partition_broadcast(128),
            )

            # ---------- xT: [feat, tok] bf16 via PE transpose (copies on ACT) ----
            xT = pp.tile([128, DC, N], bf16, tag="xT")

            def xt_group(t, g):
                # transpose feature chunks 4g..4g+3 of token tile t
                src = xc_tiles[t // 4] if g == 1 else xe_tiles[t // 4]
                pst = ps_mm.tile([128, 4, 128], bf16, tag="mm", name=f"pst{t}_{g}")
                for k in range(4):
                    nc.tensor.transpose(
                        pst[:, k, :],
                        src[:, t % 4, k * 128 : (k + 1) * 128],
                        ident[:, :],
                    )
                nc.scalar.copy(
                    xT[:, g * 4 : (g + 1) * 4, t * 128 : (t + 1) * 128], pst[:, :, :]
                )

            # ---------- qkT[f, n] = Wqk^T @ x_clsT + bqk (bias-copy on ACT) ------
            # Only (m, nh) tiles m in {0, 4} gate attention pair 0, and each
            # nh half needs just 4 token tiles of x_clsT — so attention can
            # start ~12us in.  The remaining 12 qkT steps ride inside the
            # pair-0 loop.
            qkT = pp.tile([128, 8, N], bf16)

            def qkt_proj_step(m, nh):
                ps = ps_mm.tile([128, 512], f32, tag="mm", name=f"psqk{m}_{nh}")
                for kc in range(CLS // 128):
                    nc.tensor.matmul(
                        ps[:, :],
                        lhsT=wqk_sb[:, kc, m * 128 : (m + 1) * 128],
                        rhs=xT[:, 4 + kc, nh * 512 : (nh + 1) * 512],
                        start=(kc == 0),
                        stop=(kc == CLS // 128 - 1),
                    )
                nc.vector.tensor_scalar(
                    qkT[:, m, nh * 512 : (nh + 1) * 512],
                    ps[:, :],
                    bqk_col[:, m : m + 1],
                    None,
                    op0=add,
                )

            # ---------- interleaved attention + v-projection + out-transpose ----
            v_aug = pp.tile([128, NT, HEADS * VSTRIDE], bf16)
            nc.vector.memset(v_aug[:, :, :], 1.0)  # ones cols survive under data
            out_sb = pp.tile([128, NT, D], bf16, tag="big2", name="out_sb")
            # outT shares the xT slot: xT is dead once the v-projection is done
            outT = pp.tile([128, DC, N], bf16, tag="xT", name="outT")
            exps = {}

            def vproj_step(i):
                # i in [0, 16): t-tile i%8, output half i//8
                t, nh = i % NT, i // NT
                ps = ps_mm.tile([128, 512], f32, tag="mm", name=f"psv{t}_{nh}")
                for kc in range(DC):
                    nc.tensor.matmul(
                        ps[:, :],
                        lhsT=xT[:, kc, t * 128 : (t + 1) * 128],
                        rhs=wv_sb[:, kc, nh * 512 : (nh + 1) * 512],
                        start=(kc == 0),
                        stop=(kc == DC - 1),
                    )
                dst = v_aug[:, t, nh * 4 * VSTRIDE : (nh + 1) * 4 * VSTRIDE]
                dst = dst.rearrange("p (h w) -> p h w", w=VSTRIDE)[:, :, 0:128]
                nc.vector.tensor_copy(
                    dst, ps[:, :].rearrange("p (h w) -> p h w", w=128)
                )

            def qkt_step(pair, jt):
                h0, h1 = 2 * pair, 2 * pair + 1
                pss = {
                    h: ps_s.tile([128, N], f32, tag="s", name=f"psS{h}_{jt}")
                    for h in (h0, h1)
                }
                for nh in range(2):
                    for h in (h0, h1):
                        pr = (h % 2) * 64
                        nc.tensor.matmul(
                            pss[h][:, nh * 512 : (nh + 1) * 512],
                            lhsT=qkT[pr : pr + 64, 4 + pair, jt * 128 : (jt + 1) * 128],
                            rhs=qkT[pr : pr + 64, pair, nh * 512 : (nh + 1) * 512],
                            start=True,
                            stop=True,
                        )
                for h in (h0, h1):
                    nc.scalar.activation(
                        exps[h][:, jt, :], pss[h][:, :], Exp, scale=SCALE
                    )

            def pv_step(pair, s):
                # s in [0, 16): head pair*2 + s//8, i-tile s%8
                h, it = 2 * pair + s // NT, s % NT
                pso = ps_o.tile([128, 129], f32, tag="o", name=f"psO{h}_{it}")
                for jc in range(NT):
                    nc.tensor.matmul(
                        pso[:, :],
                        lhsT=exps[h][:, jc, it * 128 : (it + 1) * 128],
                        rhs=v_aug[:, jc, h * VSTRIDE : h * VSTRIDE + 129],
                        start=(jc == 0),
                        stop=(jc == NT - 1),
                    )
                recip = sp.tile([128, 1], f32, tag="recip", name=f"rc{h}_{it}")
                nc.vector.reciprocal(recip[:, :], pso[:, 128:129])
                nc.vector.tensor_scalar(
                    out_sb[:, it, h * DV : (h + 1) * DV],
                    pso[:, 0:DV],
                    recip[:, :],
                    None,
                    op0=mult,
                )

            def outT_step(g, it):
                # transpose heads 4g..4g+3 of i-tile `it`; add bv on the way out
                pst = ps_mm.tile([128, 4, 128], bf16, tag="mm", name=f"psoT{g}_{it}")
                for k in range(4):
                    c = g * 4 + k
                    nc.tensor.transpose(
                        pst[:, k, :],
                        out_sb[:, it, c * 128 : (c + 1) * 128],
                        ident[:, :],
                    )
                for k in range(4):
                    c = g * 4 + k
                    nc.vector.tensor_scalar(
                        outT[:, c, it * 128 : (it + 1) * 128],
                        pst[:, k, :],
                        bv_col[:, c : c + 1],
                        None,
                        op0=add,
                    )

            # prologue: transposes + the 4 qkT steps that gate pair 0
            for t in range(4):
                xt_group(t, 1)
            qkt_proj_step(0, 0)
            qkt_proj_step(4, 0)
            for t in range(4, NT):
                xt_group(t, 1)
            qkt_proj_step(0, 1)
            qkt_proj_step(4, 1)

            # Pair 0 is filled by the remaining qkT-projection steps alone:
            # they need nothing beyond Wqk + x_cls, which are the only loads
            # that can have landed this early at the real ~358 GB/s per-core
            # HBM rate.  The v-projection (which needs xe + Wv halves) fills
            # the first halves of pairs 1-2; PV packs into their second
            # halves so all v_aug writes precede every PV read in program
            # order.
            QKT_REST = [(1, 0), (5, 0), (1, 1), (5, 1), (2, 0), (6, 0),
                        (2, 1), (6, 1), (3, 0), (7, 0), (3, 1), (7, 1)]
            for pair in range(HEADS // 2):
                h0, h1 = 2 * pair, 2 * pair + 1
                exps[h0] = ep.tile([128, NT, N], bf16, tag="expS", name=f"eS{h0}")
                exps[h1] = ep.tile([128, NT, N], bf16, tag="expS", name=f"eS{h1}")
                if pair == 1:
                    xt_group(0, 0)
                    xt_group(1, 0)
                for jt in range(NT):
                    qkt_step(pair, jt)
                    if pair == 0:
                        if jt < 6:
                            qkt_proj_step(*QKT_REST[2 * jt])
                            qkt_proj_step(*QKT_REST[2 * jt + 1])
                    elif pair <= 2:
                        base = 8 * (pair - 1)     # nh half for this pair
                        if jt < 3 and pair == 1:
                            xt_group(2 + 2 * jt, 0)
                            xt_group(3 + 2 * jt, 0)
                        if jt < 4:
                            vproj_step(base + 2 * jt)
                            vproj_step(base + 2 * jt + 1)
                        else:
                            for q in range(4):
                                pv_step(pair - 1, 4 * (jt - 4) + q)
                    else:
                        pv_step(pair - 1, 2 * jt)
                        pv_step(pair - 1, 2 * jt + 1)
                        outT_step(0, jt)
            for s in range(16):
                pv_step(3, s)

            # ---------- y = outT^T @ Wo + bo (outT g1 interleaved) ----------
            outT_step(1, 0)
            for mt in range(NT):
                if mt + 1 < NT:
                    outT_step(1, mt + 1)
                for nh in range(2):
                    y_tile = yp.tile([128, 512], f32, tag="y", name=f"y{mt}_{nh}")
                    ps = ps_s.tile([128, 512], f32, tag="s", name=f"psy{mt}_{nh}")
                    for kc in range(DC):
                        nc.tensor.matmul(
                            ps[:, :],
                            lhsT=outT[:, kc, mt * 128 : (mt + 1) * 128],
                            rhs=wo_sb[:, kc, nh * 512 : (nh + 1) * 512],
                            start=(kc == 0),
                            stop=(kc == DC - 1),
                        )
                    nc.vector.tensor_tensor(
                        y_tile[:, :],
                        ps[:, :],
                        bo_bc[:, nh * 512 : (nh + 1) * 512],
                        op=add,
                    )
                    nc.sync.dma_start(
                        out=y_d[mt * 128 : (mt + 1) * 128, nh * 512 : (nh + 1) * 512],
                        in_=y_tile[:, :],
                    )

    nc.finalize()
    return nc


def _get_compiled():
    global _COMPILED
    if _COMPILED is None:
        _COMPILED = _build()
    return _COMPILED


def _run(inputs: dict, trace: bool = False):
    from concourse.bass_utils import run_bass_kernel_spmd

    nc = _get_compiled()
    x = np.ascontiguousarray(np.asarray(inputs["x"], dtype=np.float32))
    shared = {
        k: np.ascontiguousarray(np.asarray(inputs[k], dtype=np.float32))
        for k in ("Wqk", "bqk", "Wv", "bv", "Wo", "bo")
    }
    in_maps = [{"x": x[b], **shared} for b in range(B)]
    res = run_bass_kernel_spmd(nc, in_maps, core_ids=list(range(B)), trace=trace)
    y = np.stack([res.results[b]["y"] for b in range(B)], axis=0)
    return y, res


def kernel(**inputs) -> np.ndarray:
    y, _ = _run(inputs, trace=False)
    return y


# revision 29
# speedup vs baseline: 1.0123x; 1.0123x over previous
"""Trainium2 Bass kernel for nn_Attention_53257594471037.

Multi-head attention layer (B=8, N=1024, embed 512 + class 512):
  qk = x[:, :, -512:] @ Wqk + bqk ; q, k = split(qk)      (8 heads, dh=64)
  v  = x @ Wv + bv                                        (8 heads, dv=128)
  out = softmax(q k^T / sqrt(64)) v                       per head
  y  = concat(out) @ Wo + bo

Sharding: data-parallel over batch — each of the 8 NeuronCores handles one
batch element end to end.  No collectives.

Per-core plan (all matmuls in bf16, fp32 accumulation in PSUM):
  - x is DMA'd (cast to bf16) and transposed on the PE into xT [feat, tok];
    the class half is loaded and transposed first since only it feeds the
    qk projection.
  - qkT[f, n] = Wqk^T @ x_clsT + bqk computed directly in transposed layout,
    which gives q^T / k^T per head ([64, 1024] slices) for free.
  - S^T[j, i] per head has j on partitions so softmax-exp runs on ACT
    straight out of PSUM; two heads are packed into the PE at once
    (K=64 row tiling).
  - The softmax denominator comes free from the PV matmul: V is augmented
    with a ones column, so out_psum[:, 128] = sum_j exp(S^T[j, i]).
  - bv is NOT added to v: softmax rows sum to one, so the bias passes
    through attention unchanged and is added per-partition after the
    out -> outT transpose instead (where vfeat sits on partitions).
  - y = outT^T @ Wo + bo.

Emission interleaves phases so the PE never waits on ACT exp, and is
ordered so every matmul's inputs have landed by the time the PE (an
in-order engine) reaches it at the real ~358 GB/s per-core HBM rate:
pair 0's exp gaps are filled by the remaining qkT-projection steps (which
need only Wqk + x_cls), the v-projection and embed-half transposes fill
the first halves of pairs 1-2, PV of pair c-1 packs into the second
halves, out-transposes of heads 0-3 ride with pair 3, and the remaining
out-transposes alternate with y-projection steps.
"""

import os

os.environ.setdefault("MYCRO_LOCAL_CACHE", "1")

import numpy as np

# --- problem constants (hardcoded; kernel.py must be self-contained) ---
B = 8
N = 1024          # tokens
D = 1024          # embed + class feature width
CLS = 512         # class width; qk projection reads x[:, :, -CLS:]
HEADS = 8
DH = 64           # per-head q/k dim
DV = 128          # per-head v dim
SCALE = DH ** -0.5
NT = N // 128     # 8 token tiles
DC = D // 128     # 8 feature chunks
VSTRIDE = 130     # per-head stride in v_aug: 128 data + 1 ones + 1 pad

_COMPILED = None  # cached compiled module so repeated kernel() calls reuse it


def _build():
    import concourse.mybir as mybir
    import concourse.tile as tile
    from concourse import bacc

    f32 = mybir.dt.float32
    bf16 = mybir.dt.bfloat16
    Exp = mybir.ActivationFunctionType.Exp
    Ident = mybir.ActivationFunctionType.Identity
    mult = mybir.AluOpType.mult
    add = mybir.AluOpType.add

    nc = bacc.Bacc(None, target_bir_lowering=False)

    x_d = nc.declare_dram_parameter("x", [N, D], f32, isOutput=False)
    wqk_d = nc.declare_dram_parameter("Wqk", [CLS, 2 * HEADS * DH], f32, isOutput=False)
    bqk_d = nc.declare_dram_parameter("bqk", [2 * HEADS * DH], f32, isOutput=False)
    wv_d = nc.declare_dram_parameter("Wv", [D, D], f32, isOutput=False)
    bv_d = nc.declare_dram_parameter("bv", [D], f32, isOutput=False)
    wo_d = nc.declare_dram_parameter("Wo", [D, D], f32, isOutput=False)
    bo_d = nc.declare_dram_parameter("bo", [D], f32, isOutput=False)
    y_d = nc.declare_dram_parameter("y", [N, D], f32, isOutput=True)

    ident_const = nc.inline_tensor(
        np.eye(128, dtype=np.float32).astype(__import__("ml_dtypes").bfloat16),
        name="identc",
    )

    with tile.TileContext(nc) as tc:
        with (
            tc.tile_pool(name="persist", bufs=1) as pp,
            tc.tile_pool(name="xin", bufs=4) as xp,
            tc.tile_pool(name="expsp", bufs=4) as ep,
            tc.tile_pool(name="small", bufs=2) as sp,
            tc.tile_pool(name="yout", bufs=2) as yp,
            tc.tile_pool(name="ps_mm", bufs=2, space="PSUM") as ps_mm,
            tc.tile_pool(name="ps_s", bufs=2, space="PSUM") as ps_s,
            tc.tile_pool(name="ps_o", bufs=2, space="PSUM") as ps_o,
        ):
            # ---------- loads; gpsimd (SWDGE, casting) queue order matters ----
            # Few LARGE DMAs: each gpsimd dma_start costs ~1us of Q7 descriptor
            # work and the queue is FIFO, so x rides in four 1MB tiles.  The
            # class half goes first (it alone feeds the qk projection); the
            # embed half loads LAST so its wait-for-slot (it reuses the class
            # tiles' slots) never stalls the weight loads behind it.
            x_r = x_d.rearrange("(t p) f -> p t f", p=128)
            xc_tiles, xe_tiles = [], []
            xc = xp.tile([128, 4, CLS], bf16, tag="xbig", name="xc0", bufs=4)
            nc.gpsimd.dma_start(out=xc[:, 0:1, :], in_=x_r[:, 0:1, CLS:D])
            nc.gpsimd.dma_start(out=xc[:, 1:4, :], in_=x_r[:, 1:4, CLS:D])
            xc_tiles.append(xc)
            wqk_sb = pp.tile([128, CLS // 128, 1024], bf16, tag="big2")
            nc.gpsimd.dma_start(
                out=wqk_sb[:, :, :], in_=wqk_d.rearrange("(c p) n -> p c n", p=128)
            )
            xc = xp.tile([128, 4, CLS], bf16, tag="xbig", name="xc1", bufs=4)
            nc.gpsimd.dma_start(out=xc[:, :, :], in_=x_r[:, 4:8, CLS:D])
            xc_tiles.append(xc)
            xe0 = xp.tile([128, 4, CLS], bf16, tag="xbig", name="xe0", bufs=4)
            nc.gpsimd.dma_start(out=xe0[:, :, :], in_=x_r[:, 0:4, 0:CLS])
            xe_tiles.append(xe0)
            wv_sb = pp.tile([128, DC, 1024], bf16)
            nc.gpsimd.dma_start(
                out=wv_sb[:, :, 0:512],
                in_=wv_d.rearrange("(c p) n -> p c n", p=128)[:, :, 0:512],
            )
            xe1 = xp.tile([128, 4, CLS], bf16, tag="xbig", name="xe1", bufs=4)
            nc.gpsimd.dma_start(out=xe1[:, :, :], in_=x_r[:, 4:8, 0:CLS])
            xe_tiles.append(xe1)
            nc.gpsimd.dma_start(
                out=wv_sb[:, :, 512:1024],
                in_=wv_d.rearrange("(c p) n -> p c n", p=128)[:, :, 512:1024],
            )
            wo_sb = pp.tile([128, DC, 1024], bf16)
            nc.gpsimd.dma_start(
                out=wo_sb[:, :, :], in_=wo_d.rearrange("(c p) n -> p c n", p=128)
            )
            # biases + identity ride the HWDGE queue, parallel to the above
            ident = pp.tile([128, 128], bf16)
            nc.sync.dma_start(out=ident[:, :], in_=ident_const[:, :])
            bqk_col = pp.tile([128, 8], f32)
            nc.sync.dma_start(
                out=bqk_col[:, :], in_=bqk_d.rearrange("(m p) -> p m", p=128)
            )
            bv_col = pp.tile([128, 8], f32)
            nc.sync.dma_start(
                out=bv_col[:, :], in_=bv_d.rearrange("(m p) -> p m", p=128)
            )
            bo_bc = pp.tile([128, D], bf16)
            nc.gpsimd.dma_start(
                out=bo_bc[:, :],
                in_=bo_d.rearrange("(o n) -> o n", o=1).partition_broadcast(128),
            )

            # ---------- xT: [feat, tok] bf16 via PE transpose (copies on ACT) ----
            xT = pp.tile([128, DC, N], bf16, tag="xT")

            def xt_group(t, g):
                # transpose feature chunks 4g..4g+3 of token tile t
                src = xc_tiles[t // 4] if g == 1 else xe_tiles[t // 4]
                pst = ps_mm.tile([128, 4, 128], bf16, tag="mm", name=f"pst{t}_{g}")
                for k in range(4):
                    nc.tensor.transpose(
                        pst[:, k, :],
                        src[:, t % 4, k * 128 : (k + 1) * 128],
                        ident[:, :],
                    )
                nc.scalar.copy(
                    xT[:, g * 4 : (g + 1) * 4, t * 128 : (t + 1) * 128], pst[:, :, :]
                )

            # ---------- qkT[f, n] = Wqk^T @ x_clsT + bqk (bias-copy on ACT) ------
            # Only (m, nh) tiles m in {0, 4} gate attention pair 0, and each
            # nh half needs just 4 token tiles of x_clsT — so attention can
            # start ~12us in.  The remaining 12 qkT steps ride inside the
            # pair-0 loop.
            qkT = pp.tile([128, 8, N], bf16)

            def qkt_proj_step(m, nh):
                ps = ps_mm.tile([128, 512], f32, tag="mm", name=f"psqk{m}_{nh}")
                for kc in range(CLS // 128):
                    nc.tensor.matmul(
                        ps[:, :],
                        lhsT=wqk_sb[:, kc, m * 128 : (m + 1) * 128],
                        rhs=xT[:, 4 + kc, nh * 512 : (nh + 1) * 512],
                        start=(kc == 0),
                        stop=(kc == CLS // 128 - 1),
                    )
                nc.vector.tensor_scalar(
                    qkT[:, m, nh * 512 : (nh + 1) * 512],
                    ps[:, :],
                    bqk_col[:, m : m + 1],
                    None,
                    op0=add,
                )

            # ---------- interleaved attention + v-projection + out-transpose ----
            v_aug = pp.tile([128, NT, HEADS * VSTRIDE], bf16)
            nc.vector.memset(v_aug[:, :, :], 1.0)  # ones cols survive under data
            out_sb = pp.tile([128, NT, D], bf16, tag="big2", name="out_sb")
            # outT shares the xT slot: xT is dead once the v-projection is done
            outT = pp.tile([128, DC, N], bf16, tag="xT", name="outT")
            exps = {}

            def vproj_step(i):
                # i in [0, 16): t-tile i%8, output half i//8
                t, nh = i % NT, i // NT
                ps = ps_mm.tile([128, 512], f32, tag="mm", name=f"psv{t}_{nh}")
                for kc in range(DC):
                    nc.tensor.matmul(
                        ps[:, :],
                        lhsT=xT[:, kc, t * 128 : (t + 1) * 128],
                        rhs=wv_sb[:, kc, nh * 512 : (nh + 1) * 512],
                        start=(kc == 0),
                        stop=(kc == DC - 1),
                    )
                dst = v_aug[:, t, nh * 4 * VSTRIDE : (nh + 1) * 4 * VSTRIDE]
                dst = dst.rearrange("p (h w) -> p h w", w=VSTRIDE)[:, :, 0:128]
                nc.vector.tensor_copy(
                    dst, ps[:, :].rearrange("p (h w) -> p h w", w=128)
                )

            def qkt_step(pair, jt):
                h0, h1 = 2 * pair, 2 * pair + 1
                pss = {
                    h: ps_s.tile([128, N], f32, tag="s", name=f"psS{h}_{jt}")
                    for h in (h0, h1)
                }
                for nh in range(2):
                    for h in (h0, h1):
                        pr = (h % 2) * 64
                        nc.tensor.matmul(
                            pss[h][:, nh * 512 : (nh + 1) * 512],
                            lhsT=qkT[pr : pr + 64, 4 + pair, jt * 128 : (jt + 1) * 128],
                            rhs=qkT[pr : pr + 64, pair, nh * 512 : (nh + 1) * 512],
                            start=True,
                            stop=True,
                        )
                for h in (h0, h1):
                    nc.scalar.activation(
                        exps[h][:, jt, :], pss[h][:, :], Exp, scale=SCALE
                    )

            def pv_step(pair, s):
                # s in [0, 16): head pair*2 + s//8, i-tile s%8
                h, it = 2 * pair + s // NT, s % NT
                pso = ps_o.tile([128, 129], f32, tag="o", name=f"psO{h}_{it}")
                for jc in range(NT):
                    nc.tensor.matmul(
                        pso[:, :],
                        lhsT=exps[h][:, jc, it * 128 : (it + 1) * 128],
                        rhs=v_aug[:, jc, h * VSTRIDE : h * VSTRIDE + 129],
                        start=(jc == 0),
                        stop=(jc == NT - 1),
                    )
                recip = sp.tile([128, 1], f32, tag="recip", name=f"rc{h}_{it}")
                nc.vector.reciprocal(recip[:, :], pso[:, 128:129])
                nc.vector.tensor_scalar(
                    out_sb[:, it, h * DV : (h + 1) * DV],
                    pso[:, 0:DV],
                    recip[:, :],
                    None,
                    op0=mult,
                )

            def outT_step(g, it):
                # transpose heads 4g..4g+3 of i-tile `it`; add bv on the way out
                pst = ps_mm.tile([128, 4, 128], bf16, tag="mm", name=f"psoT{g}_{it}")
                for k in range(4):
                    c = g * 4 + k
                    nc.tensor.transpose(
                        pst[:, k, :],
                        out_sb[:, it, c * 128 : (c + 1) * 128],
                        ident[:, :],
                    )
                for k in range(4):
                    c = g * 4 + k
                    nc.vector.tensor_scalar(
                        outT[:, c, it * 128 : (it + 1) * 128],
                        pst[:, k, :],
                        bv_col[:, c : c + 1],
                        None,
                        op0=add,
                    )

            # prologue: transposes + the 4 qkT steps that gate pair 0
            for t in range(4):
                xt_group(t, 1)
            qkt_proj_step(0, 0)
            qkt_proj_step(4, 0)
            for t in range(4, NT):
                xt_group(t, 1)
            qkt_proj_step(0, 1)
            qkt_proj_step(4, 1)

            # Pair 0 is filled by the remaining qkT-projection steps alone:
            # they need nothing beyond Wqk + x_cls, which are the only loads
            # that can have landed this early at the real ~358 GB/s per-core
            # HBM rate.  The v-projection (which needs xe + Wv halves) fills
            # the first halves of pairs 1-2; PV packs into their second
            # halves so all v_aug writes precede every PV read in program
            # order.
            QKT_REST = [(1, 0), (5, 0), (1, 1), (5, 1), (2, 0), (6, 0),
                        (2, 1), (6, 1), (3, 0), (7, 0), (3, 1), (7, 1)]
            for pair in range(HEADS // 2):
                h0, h1 = 2 * pair, 2 * pair + 1
                exps[h0] = ep.tile([128, NT, N], bf16, tag="expS", name=f"eS{h0}")
                exps[h1] = ep.tile([128, NT, N], bf16, tag="expS", name=f"eS{h1}")
                for jt in range(NT):
                    qkt_step(pair, jt)
                    if pair == 0:
                        # jt 0-5: remaining qkT-projection steps; jt 4-7:
                        # embed transposes + the first two v-proj steps ride
                        # in pair 0's ACT-bound pocket (their loads have
                        # landed by now even at real HBM rates)
                        if jt < 6:
                            qkt_proj_step(*QKT_REST[2 * jt])
                            qkt_proj_step(*QKT_REST[2 * jt + 1])
                        if jt >= 4:
                            xt_group(2 * (jt - 4), 0)
                            xt_group(2 * (jt - 4) + 1, 0)
                        if jt >= 6:
                            vproj_step(jt - 6)
                    elif pair == 1:
                        if jt < 3:
                            vproj_step(2 + 2 * jt)
                            vproj_step(3 + 2 * jt)
                        else:
                            for q in range(3):
                                s = 3 * (jt - 3) + q
                                if s < 16:
                                    pv_step(0, s)
                        if jt == 7:
                            pv_step(0, 15)
                    elif pair == 2:
                        if jt < 4:
                            vproj_step(8 + 2 * jt)
                            vproj_step(9 + 2 * jt)
                        else:
                            for q in range(4):
                                pv_step(1, 4 * (jt - 4) + q)
                    else:
                        pv_step(pair - 1, 2 * jt)
                        pv_step(pair - 1, 2 * jt + 1)
                        outT_step(0, jt)
            for s in range(16):
                pv_step(3, s)

            # ---------- y = outT^T @ Wo + bo (outT g1 interleaved) ----------
            outT_step(1, 0)
            for mt in range(NT):
                if mt + 1 < NT:
                    outT_step(1, mt + 1)
                for nh in range(2):
                    y_tile = yp.tile([128, 512], f32, tag="y", name=f"y{mt}_{nh}")
                    ps = ps_s.tile([128, 512], f32, tag="s", name=f"psy{mt}_{nh}")
                    for kc in range(DC):
                        nc.tensor.matmul(
                            ps[:, :],
                            lhsT=outT[:, kc, mt * 128 : (mt + 1) * 128],
                            rhs=wo_sb[:, kc, nh * 512 : (nh + 1) * 512],
                            start=(kc == 0),
                            stop=(kc == DC - 1),
                        )
                    nc.vector.tensor_tensor(
                        y_tile[:, :],
                        ps[:, :],
                        bo_bc[:, nh * 512 : (nh + 1) * 512],
                        op=add,
                    )
                    nc.sync.dma_start(
                        out=y_d[mt * 128 : (mt + 1) * 128, nh * 512 : (nh + 1) * 512],
                        in_=y_tile[:, :],
                    )

    nc.finalize()
    return nc


def _get_compiled():
    global _COMPILED
    if _COMPILED is None:
        _COMPILED = _build()
    return _COMPILED


def _run(inputs: dict, trace: bool = False):
    from concourse.bass_utils import run_bass_kernel_spmd

    nc = _get_compiled()
    x = np.ascontiguousarray(np.asarray(inputs["x"], dtype=np.float32))
    shared = {
        k: np.ascontiguousarray(np.asarray(inputs[k], dtype=np.float32))
        for k in ("Wqk", "bqk", "Wv", "bv", "Wo", "bo")
    }
    in_maps = [{"x": x[b], **shared} for b in range(B)]
    res = run_bass_kernel_spmd(nc, in_maps, core_ids=list(range(B)), trace=trace)
    y = np.stack([res.results[b]["y"] for b in range(B)], axis=0)
    return y, res


def kernel(**inputs) -> np.ndarray:
    y, _ = _run(inputs, trace=False)
    return y


# revision 35
# speedup vs baseline: 1.0362x; 1.0236x over previous
"""Trainium2 Bass kernel for nn_Attention_53257594471037.

Multi-head attention layer (B=8, N=1024, embed 512 + class 512):
  qk = x[:, :, -512:] @ Wqk + bqk ; q, k = split(qk)      (8 heads, dh=64)
  v  = x @ Wv + bv                                        (8 heads, dv=128)
  out = softmax(q k^T / sqrt(64)) v                       per head
  y  = concat(out) @ Wo + bo

Sharding: data-parallel over batch — each of the 8 NeuronCores handles one
batch element end to end.  No collectives.

Per-core plan (all matmuls in bf16, fp32 accumulation in PSUM):
  - x is DMA'd (cast to bf16) and transposed on the PE into xT [feat, tok];
    the class half is loaded and transposed first since only it feeds the
    qk projection.
  - qkT[f, n] = Wqk^T @ x_clsT + bqk computed directly in transposed layout,
    which gives q^T / k^T per head ([64, 1024] slices) for free.
  - S^T[j, i] per head has j on partitions so softmax-exp runs on ACT
    straight out of PSUM; two heads are packed into the PE at once
    (K=64 row tiling).
  - The softmax denominator comes free from the PV matmul: V is augmented
    with a ones column, so out_psum[:, 128] = sum_j exp(S^T[j, i]).
  - bv is NOT added to v: softmax rows sum to one, so the bias passes
    through attention unchanged and is added per-partition after the
    out -> outT transpose instead (where vfeat sits on partitions).
  - y = outT^T @ Wo + bo.

Emission interleaves phases so the PE never waits on ACT exp, and is
ordered so every matmul's inputs have landed by the time the PE (an
in-order engine) reaches it at the real ~358 GB/s per-core HBM rate:
pair 0's exp gaps are filled by the remaining qkT-projection steps (which
need only Wqk + x_cls), the v-projection and embed-half transposes fill
the first halves of pairs 1-2, PV of pair c-1 packs into the second
halves, out-transposes of heads 0-3 ride with pair 3, and the remaining
out-transposes alternate with y-projection steps.
"""

import os

os.environ.setdefault("MYCRO_LOCAL_CACHE", "1")

import numpy as np

# --- problem constants (hardcoded; kernel.py must be self-contained) ---
B = 8
N = 1024          # tokens
D = 1024          # embed + class feature width
CLS = 512         # class width; qk projection reads x[:, :, -CLS:]
HEADS = 8
DH = 64           # per-head q/k dim
DV = 128          # per-head v dim
SCALE = DH ** -0.5
NT = N // 128     # 8 token tiles
DC = D // 128     # 8 feature chunks
VSTRIDE = 130     # per-head stride in v_aug: 128 data + 1 ones + 1 pad

_COMPILED = None  # cached compiled module so repeated kernel() calls reuse it


def _build():
    import concourse.mybir as mybir
    import concourse.tile as tile
    from concourse import bacc

    f32 = mybir.dt.float32
    bf16 = mybir.dt.bfloat16
    Exp = mybir.ActivationFunctionType.Exp
    Ident = mybir.ActivationFunctionType.Identity
    mult = mybir.AluOpType.mult
    add = mybir.AluOpType.add

    nc = bacc.Bacc(None, target_bir_lowering=False)

    x_d = nc.declare_dram_parameter("x", [N, D], f32, isOutput=False)
    wqk_d = nc.declare_dram_parameter("Wqk", [CLS, 2 * HEADS * DH], f32, isOutput=False)
    bqk_d = nc.declare_dram_parameter("bqk", [2 * HEADS * DH], f32, isOutput=False)
    wv_d = nc.declare_dram_parameter("Wv", [D, D], f32, isOutput=False)
    bv_d = nc.declare_dram_parameter("bv", [D], f32, isOutput=False)
    wo_d = nc.declare_dram_parameter("Wo", [D, D], f32, isOutput=False)
    bo_d = nc.declare_dram_parameter("bo", [D], f32, isOutput=False)
    y_d = nc.declare_dram_parameter("y", [N, D], f32, isOutput=True)

    ident_const = nc.inline_tensor(
        np.eye(128, dtype=np.float32).astype(__import__("ml_dtypes").bfloat16),
        name="identc",
    )

    with tile.TileContext(nc) as tc:
        with (
            tc.tile_pool(name="persist", bufs=1) as pp,
            tc.tile_pool(name="xin", bufs=4) as xp,
            tc.tile_pool(name="expsp", bufs=4) as ep,
            tc.tile_pool(name="small", bufs=2) as sp,
            tc.tile_pool(name="yout", bufs=4) as yp,
            tc.tile_pool(name="ps_mm", bufs=2, space="PSUM") as ps_mm,
            tc.tile_pool(name="ps_s", bufs=2, space="PSUM") as ps_s,
            tc.tile_pool(name="ps_o", bufs=2, space="PSUM") as ps_o,
        ):
            # ---------- loads; gpsimd (SWDGE, casting) queue order matters ----
            # Few LARGE DMAs: each gpsimd dma_start costs ~1us of Q7 descriptor
            # work and the queue is FIFO, so x rides in four 1MB tiles.  The
            # class half goes first (it alone feeds the qk projection); the
            # embed half loads LAST so its wait-for-slot (it reuses the class
            # tiles' slots) never stalls the weight loads behind it.
            x_r = x_d.rearrange("(t p) f -> p t f", p=128)
            xc_tiles, xe_tiles = [], []
            xc = xp.tile([128, 4, CLS], bf16, tag="xbig", name="xc0", bufs=4)
            nc.gpsimd.dma_start(out=xc[:, 0:1, :], in_=x_r[:, 0:1, CLS:D])
            nc.gpsimd.dma_start(out=xc[:, 1:4, :], in_=x_r[:, 1:4, CLS:D])
            xc_tiles.append(xc)
            wqk_sb = pp.tile([128, CLS // 128, 1024], bf16, tag="big2")
            nc.gpsimd.dma_start(
                out=wqk_sb[:, :, :], in_=wqk_d.rearrange("(c p) n -> p c n", p=128)
            )
            xc = xp.tile([128, 4, CLS], bf16, tag="xbig", name="xc1", bufs=4)
            nc.gpsimd.dma_start(out=xc[:, :, :], in_=x_r[:, 4:8, CLS:D])
            xc_tiles.append(xc)
            xe0 = xp.tile([128, 4, CLS], bf16, tag="xbig", name="xe0", bufs=4)
            nc.gpsimd.dma_start(out=xe0[:, :, :], in_=x_r[:, 0:4, 0:CLS])
            xe_tiles.append(xe0)
            wv_sb = pp.tile([128, DC, 1024], bf16)
            nc.gpsimd.dma_start(
                out=wv_sb[:, :, 0:512],
                in_=wv_d.rearrange("(c p) n -> p c n", p=128)[:, :, 0:512],
            )
            xe1 = xp.tile([128, 4, CLS], bf16, tag="xbig", name="xe1", bufs=4)
            nc.gpsimd.dma_start(out=xe1[:, :, :], in_=x_r[:, 4:8, 0:CLS])
            xe_tiles.append(xe1)
            nc.gpsimd.dma_start(
                out=wv_sb[:, :, 512:1024],
                in_=wv_d.rearrange("(c p) n -> p c n", p=128)[:, :, 512:1024],
            )
            wo_sb = pp.tile([128, DC, 1024], bf16)
            nc.gpsimd.dma_start(
                out=wo_sb[:, :, :], in_=wo_d.rearrange("(c p) n -> p c n", p=128)
            )
            # biases + identity ride the HWDGE queue, parallel to the above
            ident = pp.tile([128, 128], bf16)
            nc.sync.dma_start(out=ident[:, :], in_=ident_const[:, :])
            bqk_col = pp.tile([128, 8], f32)
            nc.sync.dma_start(
                out=bqk_col[:, :], in_=bqk_d.rearrange("(m p) -> p m", p=128)
            )
            bv_col = pp.tile([128, 8], f32)
            nc.sync.dma_start(
                out=bv_col[:, :], in_=bv_d.rearrange("(m p) -> p m", p=128)
            )
            bo_bc = pp.tile([128, D], bf16)
            nc.gpsimd.dma_start(
                out=bo_bc[:, :],
                in_=bo_d.rearrange("(o n) -> o n", o=1).partition_broadcast(128),
            )

            # ---------- xT: [feat, tok] bf16 via PE transpose (copies on ACT) ----
            xT = pp.tile([128, DC, N], bf16, tag="xT")

            def xt_group(t, g):
                # transpose feature chunks 4g..4g+3 of token tile t.  The
                # class-half copies run on ACT (idle during the prologue);
                # the embed-half copies run on DVE — they execute inside
                # pair 0, where ACT is saturated by softmax-exp.
                src = xc_tiles[t // 4] if g == 1 else xe_tiles[t // 4]
                pst = ps_mm.tile([128, 4, 128], bf16, tag="mm", name=f"pst{t}_{g}")
                for k in range(4):
                    nc.tensor.transpose(
                        pst[:, k, :],
                        src[:, t % 4, k * 128 : (k + 1) * 128],
                        ident[:, :],
                    )
                eng = nc.scalar.copy if g == 1 else nc.vector.tensor_copy
                eng(
                    xT[:, g * 4 : (g + 1) * 4, t * 128 : (t + 1) * 128], pst[:, :, :]
                )

            # ---------- qkT[f, n] = Wqk^T @ x_clsT + bqk (bias-copy on ACT) ------
            # Only (m, nh) tiles m in {0, 4} gate attention pair 0, and each
            # nh half needs just 4 token tiles of x_clsT — so attention can
            # start ~12us in.  The remaining 12 qkT steps ride inside the
            # pair-0 loop.
            qkT = pp.tile([128, 8, N], bf16)

            def qkt_proj_step(m, nh):
                ps = ps_mm.tile([128, 512], f32, tag="mm", name=f"psqk{m}_{nh}")
                for kc in range(CLS // 128):
                    nc.tensor.matmul(
                        ps[:, :],
                        lhsT=wqk_sb[:, kc, m * 128 : (m + 1) * 128],
                        rhs=xT[:, 4 + kc, nh * 512 : (nh + 1) * 512],
                        start=(kc == 0),
                        stop=(kc == CLS // 128 - 1),
                    )
                nc.vector.tensor_scalar(
                    qkT[:, m, nh * 512 : (nh + 1) * 512],
                    ps[:, :],
                    bqk_col[:, m : m + 1],
                    None,
                    op0=add,
                )

            # ---------- interleaved attention + v-projection + out-transpose ----
            v_aug = pp.tile([128, NT, HEADS * VSTRIDE], bf16)
            nc.vector.memset(v_aug[:, :, :], 1.0)  # ones cols survive under data
            out_sb = pp.tile([128, NT, D], bf16, tag="big2", name="out_sb")
            # outT shares the xT slot: xT is dead once the v-projection is done
            outT = pp.tile([128, DC, N], bf16, tag="xT", name="outT")
            exps = {}

            def vproj_step(i):
                # i in [0, 16): t-tile i%8, output half i//8
                t, nh = i % NT, i // NT
                ps = ps_mm.tile([128, 512], f32, tag="mm", name=f"psv{t}_{nh}")
                for kc in range(DC):
                    nc.tensor.matmul(
                        ps[:, :],
                        lhsT=xT[:, kc, t * 128 : (t + 1) * 128],
                        rhs=wv_sb[:, kc, nh * 512 : (nh + 1) * 512],
                        start=(kc == 0),
                        stop=(kc == DC - 1),
                    )
                dst = v_aug[:, t, nh * 4 * VSTRIDE : (nh + 1) * 4 * VSTRIDE]
                dst = dst.rearrange("p (h w) -> p h w", w=VSTRIDE)[:, :, 0:128]
                nc.vector.tensor_copy(
                    dst, ps[:, :].rearrange("p (h w) -> p h w", w=128)
                )

            def qkt_step(pair, jt):
                h0, h1 = 2 * pair, 2 * pair + 1
                pss = {
                    h: ps_s.tile([128, N], f32, tag="s", name=f"psS{h}_{jt}")
                    for h in (h0, h1)
                }
                for nh in range(2):
                    for h in (h0, h1):
                        pr = (h % 2) * 64
                        nc.tensor.matmul(
                            pss[h][:, nh * 512 : (nh + 1) * 512],
                            lhsT=qkT[pr : pr + 64, 4 + pair, jt * 128 : (jt + 1) * 128],
                            rhs=qkT[pr : pr + 64, pair, nh * 512 : (nh + 1) * 512],
                            start=True,
                            stop=True,
                        )
                for h in (h0, h1):
                    nc.scalar.activation(
                        exps[h][:, jt, :], pss[h][:, :], Exp, scale=SCALE
                    )

            def pv_step(pair, s):
                # s in [0, 16): head pair*2 + s//8, i-tile s%8
                h, it = 2 * pair + s // NT, s % NT
                pso = ps_o.tile([128, 129], f32, tag="o", name=f"psO{h}_{it}")
                for jc in range(NT):
                    nc.tensor.matmul(
                        pso[:, :],
                        lhsT=exps[h][:, jc, it * 128 : (it + 1) * 128],
                        rhs=v_aug[:, jc, h * VSTRIDE : h * VSTRIDE + 129],
                        start=(jc == 0),
                        stop=(jc == NT - 1),
                    )
                recip = sp.tile([128, 1], f32, tag="recip", name=f"rc{h}_{it}")
                nc.vector.reciprocal(recip[:, :], pso[:, 128:129])
                nc.vector.tensor_scalar(
                    out_sb[:, it, h * DV : (h + 1) * DV],
                    pso[:, 0:DV],
                    recip[:, :],
                    None,
                    op0=mult,
                )

            def outT_step(g, it):
                # transpose heads 4g..4g+3 of i-tile `it`; add bv on the way out
                pst = ps_mm.tile([128, 4, 128], bf16, tag="mm", name=f"psoT{g}_{it}")
                for k in range(4):
                    c = g * 4 + k
                    nc.tensor.transpose(
                        pst[:, k, :],
                        out_sb[:, it, c * 128 : (c + 1) * 128],
                        ident[:, :],
                    )
                for k in range(4):
                    c = g * 4 + k
                    nc.vector.tensor_scalar(
                        outT[:, c, it * 128 : (it + 1) * 128],
                        pst[:, k, :],
                        bv_col[:, c : c + 1],
                        None,
                        op0=add,
                    )

            # prologue: transposes + the 4 qkT steps that gate pair 0
            for t in range(4):
                xt_group(t, 1)
            qkt_proj_step(0, 0)
            qkt_proj_step(4, 0)
            for t in range(4, NT):
                xt_group(t, 1)
            qkt_proj_step(0, 1)
            qkt_proj_step(4, 1)

            # Pair 0 is filled by the remaining qkT-projection steps alone:
            # they need nothing beyond Wqk + x_cls, which are the only loads
            # that can have landed this early at the real ~358 GB/s per-core
            # HBM rate.  The v-projection (which needs xe + Wv halves) fills
            # the first halves of pairs 1-2; PV packs into their second
            # halves so all v_aug writes precede every PV read in program
            # order.
            QKT_REST = [(1, 0), (5, 0), (1, 1), (5, 1), (2, 0), (6, 0),
                        (2, 1), (6, 1), (3, 0), (7, 0), (3, 1), (7, 1)]
            for pair in range(HEADS // 2):
                h0, h1 = 2 * pair, 2 * pair + 1
                exps[h0] = ep.tile([128, NT, N], bf16, tag="expS", name=f"eS{h0}")
                exps[h1] = ep.tile([128, NT, N], bf16, tag="expS", name=f"eS{h1}")
                for jt in range(NT):
                    qkt_step(pair, jt)
                    if pair == 0:
                        # jt 0-5: remaining qkT-projection steps; jt 4-7:
                        # embed transposes + the first two v-proj steps ride
                        # in pair 0's ACT-bound pocket (their loads have
                        # landed by now even at real HBM rates)
                        if jt < 6:
                            qkt_proj_step(*QKT_REST[2 * jt])
                            qkt_proj_step(*QKT_REST[2 * jt + 1])
                        if jt >= 4:
                            xt_group(2 * (jt - 4), 0)
                            xt_group(2 * (jt - 4) + 1, 0)
                        if jt >= 6:
                            vproj_step(jt - 6)
                    elif pair == 1:
                        if jt < 3:
                            vproj_step(2 + 2 * jt)
                            vproj_step(3 + 2 * jt)
                        else:
                            for q in range(3):
                                s = 3 * (jt - 3) + q
                                if s < 16:
                                    pv_step(0, s)
                        if jt == 7:
                            pv_step(0, 15)
                    elif pair == 2:
                        if jt < 4:
                            vproj_step(8 + 2 * jt)
                            vproj_step(9 + 2 * jt)
                        else:
                            for q in range(4):
                                pv_step(1, 4 * (jt - 4) + q)
                    else:
                        pv_step(pair - 1, 2 * jt)
                        pv_step(pair - 1, 2 * jt + 1)
                        outT_step(0, jt)
            for s in range(16):
                pv_step(3, s)

            # ---------- y = outT^T @ Wo + bo (outT g1 interleaved) ----------
            outT_step(1, 0)
            for mt in range(NT):
                if mt + 1 < NT:
                    outT_step(1, mt + 1)
                for nh in range(2):
                    y_tile = yp.tile([128, 512], f32, tag="y", name=f"y{mt}_{nh}")
                    ps = ps_s.tile([128, 512], f32, tag="s", name=f"psy{mt}_{nh}")
                    for kc in range(DC):
                        nc.tensor.matmul(
                            ps[:, :],
                            lhsT=outT[:, kc, mt * 128 : (mt + 1) * 128],
                            rhs=wo_sb[:, kc, nh * 512 : (nh + 1) * 512],
                            start=(kc == 0),
                            stop=(kc == DC - 1),
                        )
                    nc.vector.tensor_tensor(
                        y_tile[:, :],
                        ps[:, :],
                        bo_bc[:, nh * 512 : (nh + 1) * 512],
                        op=add,
                    )
                    nc.sync.dma_start(
                        out=y_d[mt * 128 : (mt + 1) * 128, nh * 512 : (nh + 1) * 512],
                        in_=y_tile[:, :],
                    )

    nc.finalize()
    return nc


def _get_compiled():
    global _COMPILED
    if _COMPILED is None:
        _COMPILED = _build()
    return _COMPILED


def _run(inputs: dict, trace: bool = False):
    from concourse.bass_utils import run_bass_kernel_spmd

    nc = _get_compiled()
    x = np.ascontiguousarray(np.asarray(inputs["x"], dtype=np.float32))
    shared = {
        k: np.ascontiguousarray(np.asarray(inputs[k], dtype=np.float32))
        for k in ("Wqk", "bqk", "Wv", "bv", "Wo", "bo")
    }
    in_maps = [{"x": x[b], **shared} for b in range(B)]
    res = run_bass_kernel_spmd(nc, in_maps, core_ids=list(range(B)), trace=trace)
    y = np.stack([res.results[b]["y"] for b in range(B)], axis=0)
    return y, res


def kernel(**inputs) -> np.ndarray:
    y, _ = _run(inputs, trace=False)
    return y


# revision 39
# speedup vs baseline: 1.0402x; 1.0038x over previous
"""Trainium2 Bass kernel for nn_Attention_53257594471037.

Multi-head attention layer (B=8, N=1024, embed 512 + class 512):
  qk = x[:, :, -512:] @ Wqk + bqk ; q, k = split(qk)      (8 heads, dh=64)
  v  = x @ Wv + bv                                        (8 heads, dv=128)
  out = softmax(q k^T / sqrt(64)) v                       per head
  y  = concat(out) @ Wo + bo

Sharding: data-parallel over batch — each of the 8 NeuronCores handles one
batch element end to end.  No collectives.

Per-core plan (all matmuls in bf16, fp32 accumulation in PSUM):
  - x is DMA'd (cast to bf16) and transposed on the PE into xT [feat, tok];
    the class half is loaded and transposed first since only it feeds the
    qk projection.
  - qkT[f, n] = Wqk^T @ x_clsT + bqk computed directly in transposed layout,
    which gives q^T / k^T per head ([64, 1024] slices) for free.
  - S^T[j, i] per head has j on partitions so softmax-exp runs on ACT
    straight out of PSUM; two heads are packed into the PE at once
    (K=64 row tiling).
  - The softmax denominator comes free from the PV matmul: V is augmented
    with a ones column, so out_psum[:, 128] = sum_j exp(S^T[j, i]).
  - bv is NOT added to v: softmax rows sum to one, so the bias passes
    through attention unchanged and is added per-partition after the
    out -> outT transpose instead (where vfeat sits on partitions).
  - y = outT^T @ Wo + bo.

Emission interleaves phases so the PE never waits on ACT exp, and is
ordered so every matmul's inputs have landed by the time the PE (an
in-order engine) reaches it at the real ~358 GB/s per-core HBM rate:
pair 0's exp gaps are filled by the remaining qkT-projection steps (which
need only Wqk + x_cls), the v-projection and embed-half transposes fill
the first halves of pairs 1-2, PV of pair c-1 packs into the second
halves, out-transposes of heads 0-3 ride with pair 3, and the remaining
out-transposes alternate with y-projection steps.
"""

import os

os.environ.setdefault("MYCRO_LOCAL_CACHE", "1")

import numpy as np

# --- problem constants (hardcoded; kernel.py must be self-contained) ---
B = 8
N = 1024          # tokens
D = 1024          # embed + class feature width
CLS = 512         # class width; qk projection reads x[:, :, -CLS:]
HEADS = 8
DH = 64           # per-head q/k dim
DV = 128          # per-head v dim
SCALE = DH ** -0.5
NT = N // 128     # 8 token tiles
DC = D // 128     # 8 feature chunks
VSTRIDE = 130     # per-head stride in v_aug: 128 data + 1 ones + 1 pad

_COMPILED = None  # cached compiled module so repeated kernel() calls reuse it


def _build():
    import concourse.mybir as mybir
    import concourse.tile as tile
    from concourse import bacc

    f32 = mybir.dt.float32
    bf16 = mybir.dt.bfloat16
    Exp = mybir.ActivationFunctionType.Exp
    Ident = mybir.ActivationFunctionType.Identity
    mult = mybir.AluOpType.mult
    add = mybir.AluOpType.add

    nc = bacc.Bacc(None, target_bir_lowering=False)

    x_d = nc.declare_dram_parameter("x", [N, D], f32, isOutput=False)
    wqk_d = nc.declare_dram_parameter("Wqk", [CLS, 2 * HEADS * DH], f32, isOutput=False)
    bqk_d = nc.declare_dram_parameter("bqk", [2 * HEADS * DH], f32, isOutput=False)
    wv_d = nc.declare_dram_parameter("Wv", [D, D], f32, isOutput=False)
    bv_d = nc.declare_dram_parameter("bv", [D], f32, isOutput=False)
    wo_d = nc.declare_dram_parameter("Wo", [D, D], f32, isOutput=False)
    bo_d = nc.declare_dram_parameter("bo", [D], f32, isOutput=False)
    y_d = nc.declare_dram_parameter("y", [N, D], f32, isOutput=True)

    ident_const = nc.inline_tensor(
        np.eye(128, dtype=np.float32).astype(__import__("ml_dtypes").bfloat16),
        name="identc",
    )

    with tile.TileContext(nc) as tc:
        with (
            tc.tile_pool(name="persist", bufs=1) as pp,
            tc.tile_pool(name="xin", bufs=4) as xp,
            tc.tile_pool(name="expsp", bufs=4) as ep,
            tc.tile_pool(name="small", bufs=2) as sp,
            tc.tile_pool(name="yout", bufs=4) as yp,
            tc.tile_pool(name="ps_mm", bufs=2, space="PSUM") as ps_mm,
            tc.tile_pool(name="ps_s", bufs=2, space="PSUM") as ps_s,
            tc.tile_pool(name="ps_o", bufs=2, space="PSUM") as ps_o,
        ):
            # ---------- loads; gpsimd (SWDGE, casting) queue order matters ----
            # Few LARGE DMAs: each gpsimd dma_start costs ~1us of Q7 descriptor
            # work and the queue is FIFO, so x rides in four 1MB tiles.  The
            # class half goes first (it alone feeds the qk projection); the
            # embed half loads LAST so its wait-for-slot (it reuses the class
            # tiles' slots) never stalls the weight loads behind it.
            x_r = x_d.rearrange("(t p) f -> p t f", p=128)
            xc_tiles, xe_tiles = [], []
            xc = xp.tile([128, 4, CLS], bf16, tag="xbig", name="xc0", bufs=4)
            nc.gpsimd.dma_start(out=xc[:, 0:1, :], in_=x_r[:, 0:1, CLS:D])
            nc.gpsimd.dma_start(out=xc[:, 1:4, :], in_=x_r[:, 1:4, CLS:D])
            xc_tiles.append(xc)
            wqk_sb = pp.tile([128, CLS // 128, 1024], bf16, tag="big2")
            nc.gpsimd.dma_start(
                out=wqk_sb[:, :, :], in_=wqk_d.rearrange("(c p) n -> p c n", p=128)
            )
            xc = xp.tile([128, 4, CLS], bf16, tag="xbig", name="xc1", bufs=4)
            nc.gpsimd.dma_start(out=xc[:, :, :], in_=x_r[:, 4:8, CLS:D])
            xc_tiles.append(xc)
            xe0 = xp.tile([128, 4, CLS], bf16, tag="xbig", name="xe0", bufs=4)
            nc.gpsimd.dma_start(out=xe0[:, :, :], in_=x_r[:, 0:4, 0:CLS])
            xe_tiles.append(xe0)
            wv_sb = pp.tile([128, DC, 1024], bf16)
            nc.gpsimd.dma_start(
                out=wv_sb[:, :, 0:512],
                in_=wv_d.rearrange("(c p) n -> p c n", p=128)[:, :, 0:512],
            )
            xe1 = xp.tile([128, 4, CLS], bf16, tag="xbig", name="xe1", bufs=4)
            nc.gpsimd.dma_start(out=xe1[:, :, :], in_=x_r[:, 4:8, 0:CLS])
            xe_tiles.append(xe1)
            nc.gpsimd.dma_start(
                out=wv_sb[:, :, 512:1024],
                in_=wv_d.rearrange("(c p) n -> p c n", p=128)[:, :, 512:1024],
            )
            wo_sb = pp.tile([128, DC, 1024], bf16)
            nc.gpsimd.dma_start(
                out=wo_sb[:, :, :], in_=wo_d.rearrange("(c p) n -> p c n", p=128)
            )
            # biases + identity ride the HWDGE queue, parallel to the above
            ident = pp.tile([128, 128], bf16)
            nc.sync.dma_start(out=ident[:, :], in_=ident_const[:, :])
            bqk_col = pp.tile([128, 8], f32)
            nc.sync.dma_start(
                out=bqk_col[:, :], in_=bqk_d.rearrange("(m p) -> p m", p=128)
            )
            bv_col = pp.tile([128, 8], f32)
            nc.sync.dma_start(
                out=bv_col[:, :], in_=bv_d.rearrange("(m p) -> p m", p=128)
            )
            bo_bc = pp.tile([128, D], bf16)
            nc.gpsimd.dma_start(
                out=bo_bc[:, :],
                in_=bo_d.rearrange("(o n) -> o n", o=1).partition_broadcast(128),
            )

            # ---------- xT: [feat, tok] bf16 via PE transpose (copies on ACT) ----
            xT = pp.tile([128, DC, N], bf16, tag="xT")

            def xt_group(t, g):
                # transpose feature chunks 4g..4g+3 of token tile t.  The
                # class-half copies run on ACT (idle during the prologue);
                # the embed-half copies run on DVE — they execute inside
                # pair 0, where ACT is saturated by softmax-exp.
                src = xc_tiles[t // 4] if g == 1 else xe_tiles[t // 4]
                pst = ps_mm.tile([128, 4, 128], bf16, tag="mm", name=f"pst{t}_{g}")
                for k in range(4):
                    nc.tensor.transpose(
                        pst[:, k, :],
                        src[:, t % 4, k * 128 : (k + 1) * 128],
                        ident[:, :],
                    )
                # class copies t0-3 on ACT (idle early); t4-7 on DVE so the
                # ACT chain never delays the qkt(.,nh=1) steps; embed copies
                # on DVE (ACT is saturated by exp when they run)
                eng = nc.scalar.copy if (g == 1 and t < 4) else nc.vector.tensor_copy
                eng(
                    xT[:, g * 4 : (g + 1) * 4, t * 128 : (t + 1) * 128], pst[:, :, :]
                )

            # ---------- qkT[f, n] = Wqk^T @ x_clsT + bqk (bias-copy on ACT) ------
            # Only (m, nh) tiles m in {0, 4} gate attention pair 0, and each
            # nh half needs just 4 token tiles of x_clsT — so attention can
            # start ~12us in.  The remaining 12 qkT steps ride inside the
            # pair-0 loop.
            qkT = pp.tile([128, 8, N], bf16)

            def qkt_proj_step(m, nh):
                ps = ps_mm.tile([128, 512], f32, tag="mm", name=f"psqk{m}_{nh}")
                for kc in range(CLS // 128):
                    nc.tensor.matmul(
                        ps[:, :],
                        lhsT=wqk_sb[:, kc, m * 128 : (m + 1) * 128],
                        rhs=xT[:, 4 + kc, nh * 512 : (nh + 1) * 512],
                        start=(kc == 0),
                        stop=(kc == CLS // 128 - 1),
                    )
                nc.vector.tensor_scalar(
                    qkT[:, m, nh * 512 : (nh + 1) * 512],
                    ps[:, :],
                    bqk_col[:, m : m + 1],
                    None,
                    op0=add,
                )

            # ---------- interleaved attention + v-projection + out-transpose ----
            v_aug = pp.tile([128, NT, HEADS * VSTRIDE], bf16)
            nc.vector.memset(v_aug[:, :, :], 1.0)  # ones cols survive under data
            out_sb = pp.tile([128, NT, D], bf16, tag="big2", name="out_sb")
            # outT shares the xT slot: xT is dead once the v-projection is done
            outT = pp.tile([128, DC, N], bf16, tag="xT", name="outT")
            exps = {}

            def vproj_step(i):
                # i in [0, 16): t-tile i%8, output half i//8
                t, nh = i % NT, i // NT
                ps = ps_mm.tile([128, 512], f32, tag="mm", name=f"psv{t}_{nh}")
                for kc in range(DC):
                    nc.tensor.matmul(
                        ps[:, :],
                        lhsT=xT[:, kc, t * 128 : (t + 1) * 128],
                        rhs=wv_sb[:, kc, nh * 512 : (nh + 1) * 512],
                        start=(kc == 0),
                        stop=(kc == DC - 1),
                    )
                dst = v_aug[:, t, nh * 4 * VSTRIDE : (nh + 1) * 4 * VSTRIDE]
                dst = dst.rearrange("p (h w) -> p h w", w=VSTRIDE)[:, :, 0:128]
                nc.vector.tensor_copy(
                    dst, ps[:, :].rearrange("p (h w) -> p h w", w=128)
                )

            def qkt_step(pair, jt):
                h0, h1 = 2 * pair, 2 * pair + 1
                pss = {
                    h: ps_s.tile([128, N], f32, tag="s", name=f"psS{h}_{jt}")
                    for h in (h0, h1)
                }
                for nh in range(2):
                    for h in (h0, h1):
                        pr = (h % 2) * 64
                        nc.tensor.matmul(
                            pss[h][:, nh * 512 : (nh + 1) * 512],
                            lhsT=qkT[pr : pr + 64, 4 + pair, jt * 128 : (jt + 1) * 128],
                            rhs=qkT[pr : pr + 64, pair, nh * 512 : (nh + 1) * 512],
                            start=True,
                            stop=True,
                        )
                for h in (h0, h1):
                    nc.scalar.activation(
                        exps[h][:, jt, :], pss[h][:, :], Exp, scale=SCALE
                    )

            def pv_step(pair, s):
                # s in [0, 16): head pair*2 + s//8, i-tile s%8
                h, it = 2 * pair + s // NT, s % NT
                pso = ps_o.tile([128, 129], f32, tag="o", name=f"psO{h}_{it}")
                for jc in range(NT):
                    nc.tensor.matmul(
                        pso[:, :],
                        lhsT=exps[h][:, jc, it * 128 : (it + 1) * 128],
                        rhs=v_aug[:, jc, h * VSTRIDE : h * VSTRIDE + 129],
                        start=(jc == 0),
                        stop=(jc == NT - 1),
                    )
                recip = sp.tile([128, 1], f32, tag="recip", name=f"rc{h}_{it}")
                nc.vector.reciprocal(recip[:, :], pso[:, 128:129])
                nc.vector.tensor_scalar(
                    out_sb[:, it, h * DV : (h + 1) * DV],
                    pso[:, 0:DV],
                    recip[:, :],
                    None,
                    op0=mult,
                )

            def outT_step(g, it):
                # transpose heads 4g..4g+3 of i-tile `it`; add bv on the way out
                pst = ps_mm.tile([128, 4, 128], bf16, tag="mm", name=f"psoT{g}_{it}")
                for k in range(4):
                    c = g * 4 + k
                    nc.tensor.transpose(
                        pst[:, k, :],
                        out_sb[:, it, c * 128 : (c + 1) * 128],
                        ident[:, :],
                    )
                for k in range(4):
                    c = g * 4 + k
                    nc.vector.tensor_scalar(
                        outT[:, c, it * 128 : (it + 1) * 128],
                        pst[:, k, :],
                        bv_col[:, c : c + 1],
                        None,
                        op0=add,
                    )

            # prologue: transposes + the 4 qkT steps that gate pair 0
            for t in range(4):
                xt_group(t, 1)
            qkt_proj_step(0, 0)
            qkt_proj_step(4, 0)
            for t in range(4, NT):
                xt_group(t, 1)
            qkt_proj_step(0, 1)
            qkt_proj_step(4, 1)

            # Pair 0 is filled by the remaining qkT-projection steps alone:
            # they need nothing beyond Wqk + x_cls, which are the only loads
            # that can have landed this early at the real ~358 GB/s per-core
            # HBM rate.  The v-projection (which needs xe + Wv halves) fills
            # the first halves of pairs 1-2; PV packs into their second
            # halves so all v_aug writes precede every PV read in program
            # order.
            QKT_REST = [(1, 0), (5, 0), (1, 1), (5, 1), (2, 0), (6, 0),
                        (2, 1), (6, 1), (3, 0), (7, 0), (3, 1), (7, 1)]
            for pair in range(HEADS // 2):
                h0, h1 = 2 * pair, 2 * pair + 1
                exps[h0] = ep.tile([128, NT, N], bf16, tag="expS", name=f"eS{h0}")
                exps[h1] = ep.tile([128, NT, N], bf16, tag="expS", name=f"eS{h1}")
                for jt in range(NT):
                    qkt_step(pair, jt)
                    if pair == 0:
                        # jt 0-5: remaining qkT-projection steps; jt 4-7:
                        # embed transposes + the first two v-proj steps ride
                        # in pair 0's ACT-bound pocket (their loads have
                        # landed by now even at real HBM rates)
                        if jt < 6:
                            qkt_proj_step(*QKT_REST[2 * jt])
                            qkt_proj_step(*QKT_REST[2 * jt + 1])
                        if jt >= 4:
                            xt_group(2 * (jt - 4), 0)
                            xt_group(2 * (jt - 4) + 1, 0)
                        if jt >= 6:
                            vproj_step(jt - 6)
                    elif pair == 1:
                        if jt < 3:
                            vproj_step(2 + 2 * jt)
                            vproj_step(3 + 2 * jt)
                        else:
                            for q in range(3):
                                s = 3 * (jt - 3) + q
                                if s < 16:
                                    pv_step(0, s)
                        if jt == 7:
                            pv_step(0, 15)
                    elif pair == 2:
                        if jt < 4:
                            vproj_step(8 + 2 * jt)
                            vproj_step(9 + 2 * jt)
                        else:
                            for q in range(4):
                                pv_step(1, 4 * (jt - 4) + q)
                    else:
                        pv_step(pair - 1, 2 * jt)
                        pv_step(pair - 1, 2 * jt + 1)
                        outT_step(0, jt)
            for s in range(16):
                pv_step(3, s)

            # ---------- y = outT^T @ Wo + bo (outT g1 interleaved) ----------
            outT_step(1, 0)
            for mt in range(NT):
                if mt + 1 < NT:
                    outT_step(1, mt + 1)
                for nh in range(2):
                    y_tile = yp.tile([128, 512], f32, tag="y", name=f"y{mt}_{nh}")
                    ps = ps_s.tile([128, 512], f32, tag="s", name=f"psy{mt}_{nh}")
                    for kc in range(DC):
                        nc.tensor.matmul(
                            ps[:, :],
                            lhsT=outT[:, kc, mt * 128 : (mt + 1) * 128],
                            rhs=wo_sb[:, kc, nh * 512 : (nh + 1) * 512],
                            start=(kc == 0),
                            stop=(kc == DC - 1),
                        )
                    nc.vector.tensor_tensor(
                        y_tile[:, :],
                        ps[:, :],
                        bo_bc[:, nh * 512 : (nh + 1) * 512],
                        op=add,
                    )
                    nc.sync.dma_start(
                        out=y_d[mt * 128 : (mt + 1) * 128, nh * 512 : (nh + 1) * 512],
                        in_=y_tile[:, :],
                    )

    nc.finalize()
    return nc


def _get_compiled():
    global _COMPILED
    if _COMPILED is None:
        _COMPILED = _build()
    return _COMPILED


def _run(inputs: dict, trace: bool = False):
    from concourse.bass_utils import run_bass_kernel_spmd

    nc = _get_compiled()
    x = np.ascontiguousarray(np.asarray(inputs["x"], dtype=np.float32))
    shared = {
        k: np.ascontiguousarray(np.asarray(inputs[k], dtype=np.float32))
        for k in ("Wqk", "bqk", "Wv", "bv", "Wo", "bo")
    }
    in_maps = [{"x": x[b], **shared} for b in range(B)]
    res = run_bass_kernel_spmd(nc, in_maps, core_ids=list(range(B)), trace=trace)
    y = np.stack([res.results[b]["y"] for b in range(B)], axis=0)
    return y, res


def kernel(**inputs) -> np.ndarray:
    y, _ = _run(inputs, trace=False)
    return y
